# revision 42
# baseline (speedup 1.0000x reference)
"""Trainium2 Bass kernel for nn_HFGA_54606214201918.

Computation (per batch element b, C=256 channels, L=4096 positions):
    xh  = (x[:, 0::2] - x[:, 1::2]) / sqrt(2)          # Haar high band  [C, L/2]
    q   = Wq @ x + bq                                  # [C, L]
    k   = Wk @ xh + bk                                 # [C, L/2]
    v   = Wv @ xh + bv                                 # [C, L/2]
    attn = softmax_over_keys((k^T q) / sqrt(C))        # [L/2, L]
    out = (v @ attn) * tanh(gate) + x

Sharding: data-parallel over batch B=8 across the 8 NeuronCores (one batch
element per core); weights are broadcast. No collectives needed.

Algebraic folds (host side):
  - G-fusion: S = k^T q / sqrt(C) = xh^T (Wk^T Wq / sqrt(C)) x + bias terms.
    The per-query term (bk^T q) is constant along the softmax (key) axis and
    cancels; bq enters via t := G x + (Wk^T bq)/sqrt(C); the Haar 1/sqrt(2)
    folds into G and Wv. The k-projection disappears entirely.
  - bv: softmax columns sum to 1, so v's bias becomes "+ gate*bv" in the
    final residual stage (free operand of scalar_tensor_tensor).
  - x8 / xh8 are quantized to fp8 on the host and DMA'd directly (1.5 MB of
    early-critical input instead of 4 MB); the fp32 x streams in afterwards
    and is only touched by the final residual add.

Device schedule: all heavy matmuls are fp8e4 + DoubleRow (K=256/instr,
~N cycles/MM, LDWEIGHTS hidden by the PE reorder window). Scales 64/16 keep
every fp8 tensor mid-range; exp applies scale=1/64 bias=-3 in the activation
so e stays far below the e4m3 max of 240. l-tile 0's attention is fused into
the per-bank arrival loop so the input load is fully hidden. A burst of
nonzero full-array matmuls at t=0 flips the PE HAM clock gate to 8/8 before
the real matmul stream begins (zero operands don't register as activity).
"""
import sys

if '/opt/trn_rl_repo' not in sys.path:
    sys.path.insert(0, '/opt/trn_rl_repo')

import numpy as np
import ml_dtypes

import concourse.bass as bass
import concourse.tile as tile
from concourse import bacc, mybir
from concourse import bass_utils

B, C, L = 8, 256, 4096
M = L // 2            # 2048 keys
P = 128               # partitions
CO = C // P           # 2 channel chunks
LB = 512              # l-tile (one PSUM bank of fp32)
NB = L // LB          # 8 l-tiles
MJ = M // P           # 16 key chunks
MA = MJ // 2          # 8 key-chunk pairs (DoubleRow)
INV_SQRT2 = 0.7071067811865476
SHIFT = 3.0           # exp(S - SHIFT): keeps e8 well under e4m3 max 240
SSCALE = 64.0         # scores computed at 64x; exp applies 1/64
LAGP = 2              # score/exp pairs emitted ahead of their consumers

F32 = mybir.dt.float32
BF16 = mybir.dt.bfloat16
F8 = mybir.dt.float8e4
E4 = ml_dtypes.float8_e4m3
AF = mybir.ActivationFunctionType
DR = mybir.MatmulPerfMode.DoubleRow
ADD = mybir.AluOpType.add

_CACHE = {}


def _build():
    nc = bacc.Bacc("TRN2", target_bir_lowering=False, debug=False, num_devices=8)

    x_d = nc.dram_tensor("x", [C, L], F32, kind="ExternalInput").ap()
    x8_d = nc.dram_tensor("x8", [P, CO, L], F8, kind="ExternalInput").ap()
    xh8_d = nc.dram_tensor("xh8", [P, CO, M], F8, kind="ExternalInput").ap()
    g2_d = nc.dram_tensor("g2", [P, CO, C], F32, kind="ExternalInput").ap()
    wv2_d = nc.dram_tensor("wv2", [P, CO, C], F32, kind="ExternalInput").ap()
    wbar_d = nc.dram_tensor("wbar2", [P, CO], F32, kind="ExternalInput").ap()
    bvg_d = nc.dram_tensor("bvg2", [P, CO], F32, kind="ExternalInput").ap()
    y_d = nc.dram_tensor("y", [C, L], F32, kind="ExternalOutput").ap()

    x3 = x_d.rearrange("(co ci) l -> ci co l", ci=P)      # [128, 2, 4096]
    y3 = y_d.rearrange("(co ci) l -> ci co l", ci=P)

    with tile.TileContext(nc) as tc:
        with tc.tile_pool(name="consts", bufs=1) as consts, \
             tc.tile_pool(name="big", bufs=1) as big, \
             tc.tile_pool(name="e", bufs=10) as e_pool, \
             tc.tile_pool(name="tmp", bufs=6) as tmp_pool, \
             tc.tile_pool(name="outp", bufs=4) as out_pool, \
             tc.tile_pool(name="pssp", bufs=2, space="PSUM") as ps_sp, \
             tc.tile_pool(name="psyh", bufs=3, space="PSUM") as ps_yh, \
             tc.tile_pool(name="psz", bufs=1, space="PSUM") as ps_z:

            # ---- warmup consts on gpsimd (earliest-starting engine) ----
            warm_w = consts.tile([P, P], BF16)     # full-array warmup lhsT
            nc.gpsimd.memset(warm_w, 1.0)
            warm_sb = consts.tile([P, LB], BF16)
            nc.gpsimd.memset(warm_sb, 1.0)

            # ---- input DMAs: early-critical fp8 x8/xh8 banks on the
            # hardware-DGE sync queue, then the fp32 x (residual only);
            # weights on the gpsimd queue.
            x_sb = big.tile([P, CO, L], F32)
            x8 = big.tile([P, CO, L], F8)
            xh8 = big.tile([P, CO, M], F8)
            g2_f = consts.tile([P, CO, C], F32)
            wv2_f = consts.tile([P, CO, C], F32)
            wbar_sb = consts.tile([P, CO], F32)
            bvg_sb = consts.tile([P, CO], F32)
            nc.sync.dma_start(out=g2_f, in_=g2_d)
            nc.sync.dma_start(out=wv2_f, in_=wv2_d)
            MB = M // NB                           # xh8 piece per bank
            for j in range(NB):
                sl = slice(j * LB, (j + 1) * LB)
                msl = slice(j * MB, (j + 1) * MB)
                if j == 0:
                    nc.sync.dma_start(out=x8[:, :, sl], in_=x8_d[:, :, sl])
                    nc.sync.dma_start(out=xh8[:, :, msl], in_=xh8_d[:, :, msl])
                else:
                    nc.sync.dma_start(out=xh8[:, :, msl], in_=xh8_d[:, :, msl])
                    nc.sync.dma_start(out=x8[:, :, sl], in_=x8_d[:, :, sl])
            for j in range(NB):
                sl = slice(j * LB, (j + 1) * LB)
                nc.sync.dma_start(out=x_sb[:, :, sl], in_=x3[:, :, sl])
            nc.gpsimd.dma_start(out=wbar_sb, in_=wbar_d)
            nc.gpsimd.dma_start(out=bvg_sb, in_=bvg_d)

            # full-array nonzero warmups: flip the PE HAM clock gate to 8/8
            # before the first real matmul (runs while the DMAs stream).
            for w in range(12):
                wp = ps_yh.tile([P, LB], F32, tag="yh", name=f"warm{w}")
                nc.tensor.matmul(wp, warm_w, warm_sb, start=True, stop=True)

            # ---- constants ----
            g2 = consts.tile([P, CO, C], F8)
            wv2 = consts.tile([P, CO, C], F8)
            nc.vector.tensor_copy(g2, g2_f)
            nc.vector.tensor_copy(wv2, wv2_f)
            ones2 = consts.tile([P, CO, 16], F8)   # DR lhsT for Z rows
            nc.vector.memset(ones2, 1.0)
            nshift = consts.tile([P, 1], F32)      # exp bias (-SHIFT)
            nc.vector.memset(nshift, -SHIFT)
            # tiny dummy exp: forces the ACT table load off the critical path
            dummy = consts.tile([1, 16], F32)
            nc.scalar.activation(dummy, warm_w[0:1, 0:16], AF.Exp)
            ones_row = consts.tile([1, P], BF16)   # 16x: recip yields 1/(16Z)
            nc.vector.memset(ones_row, 16.0)

            # ---- big persistent tensors ----
            t8 = big.tile([P, CO, L], F8)          # t'[c, l] = 64*(Gx+wbar)
            vt8 = big.tile([P, MA, 2, C], F8)      # v'[m, o] pair-interleaved

            def scores_pair(lt, a, pend):
                sl = slice(lt * LB, (lt + 1) * LB)
                sp2 = ps_sp.tile([P, CO, LB], F32, tag="sp",
                                 name=f"sp{lt}_{a}")
                for i in range(2):
                    mj = 2 * a + i
                    nc.tensor.matmul(
                        sp2[:, i, :], xh8[:, :, mj * P:(mj + 1) * P],
                        t8[:, :, sl], start=True, stop=True, perf_mode=DR)
                e2 = e_pool.tile([P, 2, LB], F8, tag="e", name=f"e{lt}_{a}")
                nc.scalar.activation(e2, sp2, AF.Exp,
                                     bias=nshift, scale=1.0 / SSCALE)
                pend[(lt, a)] = e2

            def consume_pair(a, zp, yhp, pend, lt=0):
                e2 = pend.pop((lt, a))
                for oc in range(CO):
                    nc.tensor.matmul(
                        yhp[oc], vt8[:, a, :, oc * P:(oc + 1) * P], e2,
                        start=(a == 0), stop=(a == MA - 1), perf_mode=DR)
                nc.tensor.matmul(zp, ones2[:, :, :1], e2,
                                 start=(a == 0), stop=(a == MA - 1),
                                 perf_mode=DR)

            def tile_tail(lt, zp, yhp):
                # b = 1/(16 Z) broadcast across partitions with a K=1 matmul
                # (the 1/16 in ones_row compensates the vt8 scale).
                sl = slice(lt * LB, (lt + 1) * LB)
                # Z -> SBUF (bf16), broadcast 16*Z across partitions with a
                # K=1 matmul, then one 128-lane reciprocal gives 1/(16 Z).
                zq = tmp_pool.tile([1, LB], BF16, tag="zq", name=f"zq{lt}")
                if lt == NB - 1:
                    nc.scalar.copy(zq, zp)     # scalar is idle at the end
                else:
                    nc.vector.tensor_copy(zq, zp)
                bp = ps_z.tile([P, LB], F32, tag="z", name=f"bp{lt}")
                nc.tensor.matmul(bp, ones_row, zq, start=True, stop=True)
                b_sb = tmp_pool.tile([P, LB], F32, tag="bsb")
                nc.vector.reciprocal_approx_fast(out=b_sb, in_=bp)
                o_sb = out_pool.tile([P, CO, LB], F32, tag="o")
                HB = LB // 2
                parts = [(0, 0), (1, 0), (0, 1), (1, 1)] \
                    if lt == NB - 1 else [(oc, None) for oc in range(CO)]
                for oc, lh in parts:
                    ls = slice(0, LB) if lh is None else \
                        slice(lh * HB, (lh + 1) * HB)
                    gsl = slice(lt * LB + ls.start, lt * LB + ls.stop)
                    u_sb = tmp_pool.tile([P, LB], F32, tag="t")
                    nc.vector.tensor_mul(u_sb[:, ls], yhp[oc][:, ls],
                                         b_sb[:, ls])
                    nc.vector.scalar_tensor_tensor(
                        out=o_sb[:, oc, ls], in0=u_sb[:, ls],
                        scalar=bvg_sb[:, oc:oc + 1],
                        in1=x_sb[:, oc, gsl], op0=ADD, op1=ADD)
                    eng = nc.sync if (lt == NB - 1 or (lt + oc) % 2) \
                        else nc.gpsimd
                    eng.dma_start(out=y3[:, oc, gsl], in_=o_sb[:, oc, ls])

            # ---- bank loop: projections + l-tile 0 attention ride the load
            def t_proj(j):
                # t projection for bank j (DR, K=256); drains split across
                # DVE and the scalar engine (Identity+bias), both fp8 out.
                sl = slice(j * LB, (j + 1) * LB)
                tp2 = ps_sp.tile([P, CO, LB], F32, tag="sp", name=f"tp{j}")
                for oc in range(CO):
                    nc.tensor.matmul(tp2[:, oc, :],
                                     g2[:, :, oc * P:(oc + 1) * P],
                                     x8[:, :, sl], start=True, stop=True,
                                     perf_mode=DR)
                if j == 0:
                    nc.scalar.add(t8[:, 0, sl], tp2[:, 0, :],
                                  wbar_sb[:, 0:1])
                    nc.scalar.add(t8[:, 1, sl], tp2[:, 1, :],
                                  wbar_sb[:, 1:2])
                else:
                    nc.vector.tensor_scalar_add(t8[:, 0, sl], tp2[:, 0, :],
                                                wbar_sb[:, 0:1])
                    nc.vector.tensor_scalar_add(t8[:, 1, sl], tp2[:, 1, :],
                                                wbar_sb[:, 1:2])

            def v_proj4(q):
                # v projection, four key chunks 4q..4q+3 -> vt8 pairs 2q,2q+1
                # (one 2-bank PSUM tile, one 1024-wide drain)
                vp2 = ps_sp.tile([P, CO, LB], F32, tag="sp", name=f"vp{q}")
                vflat = vp2.rearrange("p c l -> p (c l)")
                for k in range(4):
                    mj = 4 * q + k
                    nc.tensor.matmul(vflat[:, k * C:(k + 1) * C],
                                     xh8[:, :, mj * P:(mj + 1) * P],
                                     wv2, start=True, stop=True, perf_mode=DR)
                vdst = vt8[:, 2 * q:2 * q + 2, :, :]
                nc.vector.tensor_copy(
                    vdst.rearrange("p a i c -> p (a i c)"), vflat)

            # ---- global pair-stream: 64 score/exp pairs, consumers LAGP
            # behind, tile tails fired inline; bank-0 work and the t/v
            # projections ride the stream where each engine has slack.
            NP = NB * MA
            pend, zps, yhps = {}, {}, {}
            for g in range(NP + LAGP):
                if g < NP:
                    lt, a = divmod(g, MA)
                    if lt == 0 and a == 0:
                        t_proj(0)
                        v_proj4(0)          # vt8 pairs 0-1
                        v_proj4(1)          # vt8 pairs 2-3
                    scores_pair(lt, a, pend)
                    if lt == 0 and a == 1:
                        v_proj4(2)          # vt8 pairs 4-5 (slack slot)
                    if lt == 0 and a == 2:
                        v_proj4(3)          # vt8 pairs 6-7
                    if a == 4 and lt + 1 < NB:
                        t_proj(lt + 1)      # t8 for the NEXT tile
                c = g - LAGP
                if c >= 0:
                    lt, a = divmod(c, MA)
                    if a == 0:
                        zps[lt] = ps_z.tile([P, LB], F32, tag="z",
                                            name=f"zp{lt}")[0:1, :]
                        yhps[lt] = [ps_yh.tile([P, LB], F32, tag="yh",
                                               name=f"yh{lt}_{i}")
                                    for i in range(CO)]
                    consume_pair(a, zps[lt], yhps[lt], pend, lt)
                    if a == MA - 1:
                        tile_tail(lt, zps.pop(lt), yhps.pop(lt))

    nc.compile()
    return nc


def _get_nc():
    if "nc" not in _CACHE:
        _CACHE["nc"] = _build()
    return _CACHE["nc"]


def _chunk_pairs(a):   # [d, c] -> [di, dj, c] with d = dj*128 + di
    return np.ascontiguousarray(
        a.reshape(CO, P, -1).transpose(1, 0, 2))


def kernel(x, Wq, bq, Wk, bk, Wv, bv, attn_gate, _run_kwargs=None):
    x = np.asarray(x, dtype=np.float32)
    Wq = np.asarray(Wq, dtype=np.float32)
    Wk = np.asarray(Wk, dtype=np.float32)
    Wv = np.asarray(Wv, dtype=np.float32)
    bq = np.asarray(bq, dtype=np.float32)
    bv = np.asarray(bv, dtype=np.float32)
    gate = float(np.tanh(np.asarray(attn_gate, dtype=np.float64))[0])

    s = 1.0 / np.sqrt(np.float64(C))
    sc_s = np.float64(SSCALE) * INV_SQRT2 * s          # scores-path scale
    # G-fusion: t = (Wk^T Wq) x + Wk^T bq, scaled by 64/sqrt(2C); bk cancels.
    Gp = (Wk.astype(np.float64).T @ Wq.astype(np.float64)) * sc_s
    wbar = (Wk.astype(np.float64).T @ bq.astype(np.float64)) * sc_s
    wvp = Wv.astype(np.float64).T * (INV_SQRT2 * gate * 16.0)

    g2 = _chunk_pairs(Gp.T).astype(np.float32)         # lhsT[d, c] = Gp[c, d]
    wv2 = _chunk_pairs(wvp).astype(np.float32)         # rhs[c, o]
    wbar2 = np.ascontiguousarray(wbar.reshape(CO, P).T).astype(np.float32)
    bvg2 = np.ascontiguousarray(
        (bv.astype(np.float64) * gate).reshape(CO, P).T).astype(np.float32)

    def q8(a):   # TRN fp8e4 (ml_dtypes e4m3, max +-240)
        return np.clip(a, -240.0, 240.0).astype(E4)

    nc = _get_nc()
    in_maps = []
    for b in range(B):
        xb = x[b]
        x8 = np.ascontiguousarray(_chunk_pairs(q8(xb)))
        xh8 = np.ascontiguousarray(_chunk_pairs(q8(xb[:, 0::2] - xb[:, 1::2])))
        in_maps.append({
            "x": np.ascontiguousarray(xb), "x8": x8, "xh8": xh8,
            "g2": g2, "wv2": wv2, "wbar2": wbar2, "bvg2": bvg2,
        })
    res = bass_utils.run_bass_kernel_spmd(
        nc, in_maps, core_ids=list(range(B)), **(_run_kwargs or {}))
    out = np.stack([res.results[b]["y"] for b in range(B)]).astype(np.float32)
    if _run_kwargs:
        kernel.last_results = res
    return out


# revision 43
# speedup vs baseline: 1.1564x; 1.1564x over previous
"""Trainium2 Bass kernel for nn_HFGA_54606214201918.

Computation (per batch element b, C=256 channels, L=4096 positions):
    xh  = (x[:, 0::2] - x[:, 1::2]) / sqrt(2)          # Haar high band  [C, L/2]
    q   = Wq @ x + bq                                  # [C, L]
    k   = Wk @ xh + bk                                 # [C, L/2]
    v   = Wv @ xh + bv                                 # [C, L/2]
    attn = softmax_over_keys((k^T q) / sqrt(C))        # [L/2, L]
    out = (v @ attn) * tanh(gate) + x

Sharding: data-parallel over batch B=8 across the 8 NeuronCores (one batch
element per core); weights are broadcast. No collectives needed.

Algebraic folds (host side):
  - G-fusion: S = k^T q / sqrt(C) = xh^T (Wk^T Wq / sqrt(C)) x + bias terms.
    The per-query term (bk^T q) is constant along the softmax (key) axis and
    cancels; bq enters via t := G x + (Wk^T bq)/sqrt(C); the Haar 1/sqrt(2)
    folds into G and Wv. The k-projection disappears entirely.
  - bv: softmax columns sum to 1, so v's bias becomes "+ gate*bv" in the
    final residual stage (free operand of scalar_tensor_tensor).
  - x8 / xh8 are quantized to fp8 on the host and DMA'd directly (1.5 MB of
    early-critical input instead of 4 MB); the fp32 x streams in afterwards
    and is only touched by the final residual add.

Device schedule: all heavy matmuls are fp8e4 + DoubleRow (K=256/instr,
~N cycles/MM, LDWEIGHTS hidden by the PE reorder window). Scales 64/16 keep
every fp8 tensor mid-range; exp applies scale=1/64 bias=-3 in the activation
so e stays far below the e4m3 max of 240. l-tile 0's attention is fused into
the per-bank arrival loop so the input load is fully hidden. A burst of
nonzero full-array matmuls at t=0 flips the PE HAM clock gate to 8/8 before
the real matmul stream begins (zero operands don't register as activity).
"""
import sys

if '/opt/trn_rl_repo' not in sys.path:
    sys.path.insert(0, '/opt/trn_rl_repo')

import numpy as np
import ml_dtypes

import concourse.bass as bass
import concourse.tile as tile
from concourse import bacc, mybir
from concourse import bass_utils

B, C, L = 8, 256, 4096
M = L // 2            # 2048 keys
P = 128               # partitions
CO = C // P           # 2 channel chunks
LB = 512              # l-tile (one PSUM bank of fp32)
NB = L // LB          # 8 l-tiles
MJ = M // P           # 16 key chunks
MA = MJ // 2          # 8 key-chunk pairs (DoubleRow)
INV_SQRT2 = 0.7071067811865476
SHIFT = 3.0           # exp(S - SHIFT): keeps e8 well under e4m3 max 240
SSCALE = 64.0         # scores computed at 64x; exp applies 1/64
LAGP = 2              # score/exp pairs emitted ahead of their consumers

F32 = mybir.dt.float32
BF16 = mybir.dt.bfloat16
F8 = mybir.dt.float8e4
E4 = ml_dtypes.float8_e4m3
AF = mybir.ActivationFunctionType
DR = mybir.MatmulPerfMode.DoubleRow
ADD = mybir.AluOpType.add

_CACHE = {}


def _build():
    nc = bacc.Bacc("TRN2", target_bir_lowering=False, debug=False, num_devices=8)

    x_d = nc.dram_tensor("x", [C, L], F32, kind="ExternalInput").ap()
    x8_d = nc.dram_tensor("x8", [P, CO, L], F8, kind="ExternalInput").ap()
    xh8_d = nc.dram_tensor("xh8", [P, CO, M], F8, kind="ExternalInput").ap()
    g2_d = nc.dram_tensor("g2", [P, CO, C], F32, kind="ExternalInput").ap()
    wv2_d = nc.dram_tensor("wv2", [P, CO, C], F32, kind="ExternalInput").ap()
    wbar_d = nc.dram_tensor("wbar2", [P, CO], F32, kind="ExternalInput").ap()
    bvg_d = nc.dram_tensor("bvg2", [P, CO], F32, kind="ExternalInput").ap()
    y_d = nc.dram_tensor("y", [C, L], F32, kind="ExternalOutput").ap()

    x3 = x_d.rearrange("(co ci) l -> ci co l", ci=P)      # [128, 2, 4096]
    y3 = y_d.rearrange("(co ci) l -> ci co l", ci=P)

    with tile.TileContext(nc) as tc:
        with tc.tile_pool(name="consts", bufs=1) as consts, \
             tc.tile_pool(name="big", bufs=1) as big, \
             tc.tile_pool(name="e", bufs=10) as e_pool, \
             tc.tile_pool(name="tmp", bufs=6) as tmp_pool, \
             tc.tile_pool(name="outp", bufs=4) as out_pool, \
             tc.tile_pool(name="pssp", bufs=2, space="PSUM") as ps_sp, \
             tc.tile_pool(name="psyh", bufs=3, space="PSUM") as ps_yh, \
             tc.tile_pool(name="psz", bufs=1, space="PSUM") as ps_z:

            # ---- warmup consts on gpsimd (earliest-starting engine) ----
            warm_w = consts.tile([P, P], BF16)     # full-array warmup lhsT
            nc.gpsimd.memset(warm_w, 1.0)
            warm_sb = consts.tile([P, LB], BF16)
            nc.gpsimd.memset(warm_sb, 1.0)

            # ---- input DMAs: early-critical fp8 x8/xh8 banks on the
            # hardware-DGE sync queue, then the fp32 x (residual only);
            # weights on the gpsimd queue.
            x_sb = big.tile([P, CO, L], F32)
            x8 = big.tile([P, CO, L], F8)
            xh8 = big.tile([P, CO, M], F8)
            g2_f = consts.tile([P, CO, C], F32)
            wv2_f = consts.tile([P, CO, C], F32)
            wbar_sb = consts.tile([P, CO], F32)
            bvg_sb = consts.tile([P, CO], F32)
            nc.sync.dma_start(out=g2_f, in_=g2_d)
            nc.sync.dma_start(out=wv2_f, in_=wv2_d)
            MB = M // NB                           # xh8 piece per bank
            for j in range(NB):
                sl = slice(j * LB, (j + 1) * LB)
                msl = slice(j * MB, (j + 1) * MB)
                if j == 0:
                    nc.sync.dma_start(out=x8[:, :, sl], in_=x8_d[:, :, sl])
                    nc.sync.dma_start(out=xh8[:, :, msl], in_=xh8_d[:, :, msl])
                else:
                    nc.sync.dma_start(out=xh8[:, :, msl], in_=xh8_d[:, :, msl])
                    nc.sync.dma_start(out=x8[:, :, sl], in_=x8_d[:, :, sl])
            for j in range(NB):
                sl = slice(j * LB, (j + 1) * LB)
                nc.sync.dma_start(out=x_sb[:, :, sl], in_=x3[:, :, sl])
            nc.gpsimd.dma_start(out=wbar_sb, in_=wbar_d)
            nc.gpsimd.dma_start(out=bvg_sb, in_=bvg_d)

            # full-array nonzero warmups: flip the PE HAM clock gate to 8/8
            # before the first real matmul (runs while the DMAs stream).
            for w in range(12):
                wp = ps_yh.tile([P, LB], F32, tag="yh", name=f"warm{w}")
                nc.tensor.matmul(wp, warm_w, warm_sb, start=True, stop=True)

            # ---- constants ----
            g2 = consts.tile([P, CO, C], F8)
            wv2 = consts.tile([P, CO, C], F8)
            nc.vector.tensor_copy(g2, g2_f)
            nc.vector.tensor_copy(wv2, wv2_f)
            ones2 = consts.tile([P, CO, 16], F8)   # DR lhsT for Z rows
            nc.vector.memset(ones2, 1.0)
            nshift = consts.tile([P, 1], F32)      # exp bias (-SHIFT)
            nc.vector.memset(nshift, -SHIFT)
            # tiny dummy exp: forces the ACT table load off the critical path
            dummy = consts.tile([1, 16], F32)
            nc.scalar.activation(dummy, warm_w[0:1, 0:16], AF.Exp)
            ones_row = consts.tile([1, P], BF16)   # 16x: recip yields 1/(16Z)
            nc.vector.memset(ones_row, 16.0)

            # ---- big persistent tensors ----
            t8 = big.tile([P, CO, L], F8)          # t'[c, l] = 64*(Gx+wbar)
            vt8 = big.tile([P, MA, 2, C], F8)      # v'[m, o] pair-interleaved

            def scores_pair(lt, a, pend):
                sl = slice(lt * LB, (lt + 1) * LB)
                sp2 = ps_sp.tile([P, CO, LB], F32, tag="sp",
                                 name=f"sp{lt}_{a}")
                for i in range(2):
                    mj = 2 * a + i
                    nc.tensor.matmul(
                        sp2[:, i, :], xh8[:, :, mj * P:(mj + 1) * P],
                        t8[:, :, sl], start=True, stop=True, perf_mode=DR)
                e2 = e_pool.tile([P, 2, LB], F8, tag="e", name=f"e{lt}_{a}")
                nc.scalar.activation(e2, sp2, AF.Exp,
                                     bias=nshift, scale=1.0 / SSCALE)
                pend[(lt, a)] = e2

            def consume_pair(a, zp, yhp, pend, lt=0):
                e2 = pend.pop((lt, a))
                for oc in range(CO):
                    nc.tensor.matmul(
                        yhp[oc], vt8[:, a, :, oc * P:(oc + 1) * P], e2,
                        start=(a == 0), stop=(a == MA - 1), perf_mode=DR)
                nc.tensor.matmul(zp, ones2[:, :, :1], e2,
                                 start=(a == 0), stop=(a == MA - 1),
                                 perf_mode=DR)

            def tile_tail(lt, zp, yhp):
                # b = 1/(16 Z) broadcast across partitions with a K=1 matmul
                # (the 1/16 in ones_row compensates the vt8 scale).
                sl = slice(lt * LB, (lt + 1) * LB)
                # Z -> SBUF (bf16), broadcast 16*Z across partitions with a
                # K=1 matmul, then one 128-lane reciprocal gives 1/(16 Z).
                zq = tmp_pool.tile([1, LB], BF16, tag="zq", name=f"zq{lt}")
                if lt == NB - 1:
                    nc.scalar.copy(zq, zp)     # scalar is idle at the end
                else:
                    nc.vector.tensor_copy(zq, zp)
                bp = ps_z.tile([P, LB], F32, tag="z", name=f"bp{lt}")
                nc.tensor.matmul(bp, ones_row, zq, start=True, stop=True)
                b_sb = tmp_pool.tile([P, LB], F32, tag="bsb")
                nc.vector.reciprocal_approx_fast(out=b_sb, in_=bp)
                o_sb = out_pool.tile([P, CO, LB], F32, tag="o")
                HB = LB // 2
                parts = [(0, 0), (1, 0), (0, 1), (1, 1)] \
                    if lt == NB - 1 else [(oc, None) for oc in range(CO)]
                for oc, lh in parts:
                    ls = slice(0, LB) if lh is None else \
                        slice(lh * HB, (lh + 1) * HB)
                    gsl = slice(lt * LB + ls.start, lt * LB + ls.stop)
                    u_sb = tmp_pool.tile([P, LB], F32, tag="t")
                    nc.vector.tensor_mul(u_sb[:, ls], yhp[oc][:, ls],
                                         b_sb[:, ls])
                    nc.vector.scalar_tensor_tensor(
                        out=o_sb[:, oc, ls], in0=u_sb[:, ls],
                        scalar=bvg_sb[:, oc:oc + 1],
                        in1=x_sb[:, oc, gsl], op0=ADD, op1=ADD)
                    eng = nc.sync if (lt == NB - 1 or (lt + oc) % 2) \
                        else nc.gpsimd
                    eng.dma_start(out=y3[:, oc, gsl], in_=o_sb[:, oc, ls])

            # ---- bank loop: projections + l-tile 0 attention ride the load
            def t_proj(j):
                # t projection for bank j (DR, K=256); drains split across
                # DVE and the scalar engine (Identity+bias), both fp8 out.
                sl = slice(j * LB, (j + 1) * LB)
                tp2 = ps_sp.tile([P, CO, LB], F32, tag="sp", name=f"tp{j}")
                for oc in range(CO):
                    nc.tensor.matmul(tp2[:, oc, :],
                                     g2[:, :, oc * P:(oc + 1) * P],
                                     x8[:, :, sl], start=True, stop=True,
                                     perf_mode=DR)
                if j == 0:
                    nc.scalar.add(t8[:, 0, sl], tp2[:, 0, :],
                                  wbar_sb[:, 0:1])
                    nc.scalar.add(t8[:, 1, sl], tp2[:, 1, :],
                                  wbar_sb[:, 1:2])
                else:
                    nc.vector.tensor_scalar_add(t8[:, 0, sl], tp2[:, 0, :],
                                                wbar_sb[:, 0:1])
                    nc.vector.tensor_scalar_add(t8[:, 1, sl], tp2[:, 1, :],
                                                wbar_sb[:, 1:2])

            def v_proj4(q):
                # v projection, four key chunks 4q..4q+3 -> vt8 pairs 2q,2q+1
                # (one 2-bank PSUM tile, one 1024-wide drain)
                vp2 = ps_sp.tile([P, CO, LB], F32, tag="sp", name=f"vp{q}")
                vflat = vp2.rearrange("p c l -> p (c l)")
                for k in range(4):
                    mj = 4 * q + k
                    nc.tensor.matmul(vflat[:, k * C:(k + 1) * C],
                                     xh8[:, :, mj * P:(mj + 1) * P],
                                     wv2, start=True, stop=True, perf_mode=DR)
                vdst = vt8[:, 2 * q:2 * q + 2, :, :]
                nc.vector.tensor_copy(
                    vdst.rearrange("p a i c -> p (a i c)"), vflat)

            # ---- global pair-stream: 64 score/exp pairs, consumers LAGP
            # behind, tile tails fired inline; bank-0 work and the t/v
            # projections ride the stream where each engine has slack.
            NP = NB * MA
            pend, zps, yhps = {}, {}, {}
            for g in range(NP + LAGP):
                if g < NP:
                    lt, a = divmod(g, MA)
                    if lt == 0 and a == 0:
                        t_proj(0)
                        v_proj4(0)          # vt8 pairs 0-1
                        v_proj4(1)          # vt8 pairs 2-3
                    scores_pair(lt, a, pend)
                    if lt == 0 and a == 1:
                        v_proj4(2)          # vt8 pairs 4-5 (slack slot)
                    if lt == 0 and a == 2:
                        v_proj4(3)          # vt8 pairs 6-7
                    if a == 1 and lt + 1 < NB:
                        t_proj(lt + 1)      # t8 for the NEXT tile
                c = g - LAGP
                if c >= 0:
                    lt, a = divmod(c, MA)
                    if a == 0:
                        zps[lt] = ps_z.tile([P, LB], F32, tag="z",
                                            name=f"zp{lt}")[0:1, :]
                        yhps[lt] = [ps_yh.tile([P, LB], F32, tag="yh",
                                               name=f"yh{lt}_{i}")
                                    for i in range(CO)]
                    consume_pair(a, zps[lt], yhps[lt], pend, lt)
                    if a == MA - 1:
                        tile_tail(lt, zps.pop(lt), yhps.pop(lt))

    nc.compile()
    return nc


def _get_nc():
    if "nc" not in _CACHE:
        _CACHE["nc"] = _build()
    return _CACHE["nc"]


def _chunk_pairs(a):   # [d, c] -> [di, dj, c] with d = dj*128 + di
    return np.ascontiguousarray(
        a.reshape(CO, P, -1).transpose(1, 0, 2))


def kernel(x, Wq, bq, Wk, bk, Wv, bv, attn_gate, _run_kwargs=None):
    x = np.asarray(x, dtype=np.float32)
    Wq = np.asarray(Wq, dtype=np.float32)
    Wk = np.asarray(Wk, dtype=np.float32)
    Wv = np.asarray(Wv, dtype=np.float32)
    bq = np.asarray(bq, dtype=np.float32)
    bv = np.asarray(bv, dtype=np.float32)
    gate = float(np.tanh(np.asarray(attn_gate, dtype=np.float64))[0])

    s = 1.0 / np.sqrt(np.float64(C))
    sc_s = np.float64(SSCALE) * INV_SQRT2 * s          # scores-path scale
    # G-fusion: t = (Wk^T Wq) x + Wk^T bq, scaled by 64/sqrt(2C); bk cancels.
    Gp = (Wk.astype(np.float64).T @ Wq.astype(np.float64)) * sc_s
    wbar = (Wk.astype(np.float64).T @ bq.astype(np.float64)) * sc_s
    wvp = Wv.astype(np.float64).T * (INV_SQRT2 * gate * 16.0)

    g2 = _chunk_pairs(Gp.T).astype(np.float32)         # lhsT[d, c] = Gp[c, d]
    wv2 = _chunk_pairs(wvp).astype(np.float32)         # rhs[c, o]
    wbar2 = np.ascontiguousarray(wbar.reshape(CO, P).T).astype(np.float32)
    bvg2 = np.ascontiguousarray(
        (bv.astype(np.float64) * gate).reshape(CO, P).T).astype(np.float32)

    def q8(a):   # TRN fp8e4 (ml_dtypes e4m3, max +-240)
        return np.clip(a, -240.0, 240.0).astype(E4)

    nc = _get_nc()
    in_maps = []
    for b in range(B):
        xb = x[b]
        x8 = np.ascontiguousarray(_chunk_pairs(q8(xb)))
        xh8 = np.ascontiguousarray(_chunk_pairs(q8(xb[:, 0::2] - xb[:, 1::2])))
        in_maps.append({
            "x": np.ascontiguousarray(xb), "x8": x8, "xh8": xh8,
            "g2": g2, "wv2": wv2, "wbar2": wbar2, "bvg2": bvg2,
        })
    res = bass_utils.run_bass_kernel_spmd(
        nc, in_maps, core_ids=list(range(B)), **(_run_kwargs or {}))
    out = np.stack([res.results[b]["y"] for b in range(B)]).astype(np.float32)
    if _run_kwargs:
        kernel.last_results = res
    return out


# revision 44
# speedup vs baseline: 1.1760x; 1.0169x over previous
"""Trainium2 Bass kernel for nn_HFGA_54606214201918.

Computation (per batch element b, C=256 channels, L=4096 positions):
    xh  = (x[:, 0::2] - x[:, 1::2]) / sqrt(2)          # Haar high band  [C, L/2]
    q   = Wq @ x + bq                                  # [C, L]
    k   = Wk @ xh + bk                                 # [C, L/2]
    v   = Wv @ xh + bv                                 # [C, L/2]
    attn = softmax_over_keys((k^T q) / sqrt(C))        # [L/2, L]
    out = (v @ attn) * tanh(gate) + x

Sharding: data-parallel over batch B=8 across the 8 NeuronCores (one batch
element per core); weights are broadcast. No collectives needed.

Algebraic folds (host side):
  - G-fusion: S = k^T q / sqrt(C) = xh^T (Wk^T Wq / sqrt(C)) x + bias terms.
    The per-query term (bk^T q) is constant along the softmax (key) axis and
    cancels; bq enters via t := G x + (Wk^T bq)/sqrt(C); the Haar 1/sqrt(2)
    folds into G and Wv. The k-projection disappears entirely.
  - bv: softmax columns sum to 1, so v's bias becomes "+ gate*bv" in the
    final residual stage (free operand of scalar_tensor_tensor).
  - x8 / xh8 are quantized to fp8 on the host and DMA'd directly (1.5 MB of
    early-critical input instead of 4 MB); the fp32 x streams in afterwards
    and is only touched by the final residual add.

Device schedule: all heavy matmuls are fp8e4 + DoubleRow (K=256/instr,
~N cycles/MM, LDWEIGHTS hidden by the PE reorder window). Scales 64/16 keep
every fp8 tensor mid-range; exp applies scale=1/64 bias=-3 in the activation
so e stays far below the e4m3 max of 240. l-tile 0's attention is fused into
the per-bank arrival loop so the input load is fully hidden. A burst of
nonzero full-array matmuls at t=0 flips the PE HAM clock gate to 8/8 before
the real matmul stream begins (zero operands don't register as activity).
"""
import sys

if '/opt/trn_rl_repo' not in sys.path:
    sys.path.insert(0, '/opt/trn_rl_repo')

import numpy as np
import ml_dtypes

import concourse.bass as bass
import concourse.tile as tile
from concourse import bacc, mybir
from concourse import bass_utils

B, C, L = 8, 256, 4096
M = L // 2            # 2048 keys
P = 128               # partitions
CO = C // P           # 2 channel chunks
LB = 512              # l-tile (one PSUM bank of fp32)
NB = L // LB          # 8 l-tiles
MJ = M // P           # 16 key chunks
MA = MJ // 2          # 8 key-chunk pairs (DoubleRow)
INV_SQRT2 = 0.7071067811865476
SHIFT = 3.0           # exp(S - SHIFT): keeps e8 well under e4m3 max 240
SSCALE = 64.0         # scores computed at 64x; exp applies 1/64
LAGP = 2              # score/exp pairs emitted ahead of their consumers

F32 = mybir.dt.float32
BF16 = mybir.dt.bfloat16
F8 = mybir.dt.float8e4
E4 = ml_dtypes.float8_e4m3
AF = mybir.ActivationFunctionType
DR = mybir.MatmulPerfMode.DoubleRow
ADD = mybir.AluOpType.add

_CACHE = {}


def _build():
    nc = bacc.Bacc("TRN2", target_bir_lowering=False, debug=False, num_devices=8)

    x_d = nc.dram_tensor("x", [C, L], F32, kind="ExternalInput").ap()
    x8_d = nc.dram_tensor("x8", [P, CO, L], F8, kind="ExternalInput").ap()
    xh8_d = nc.dram_tensor("xh8", [P, CO, M], F8, kind="ExternalInput").ap()
    g2_d = nc.dram_tensor("g2", [P, CO, C], F32, kind="ExternalInput").ap()
    wv2_d = nc.dram_tensor("wv2", [P, CO, C], F32, kind="ExternalInput").ap()
    wbar_d = nc.dram_tensor("wbar2", [P, CO], F32, kind="ExternalInput").ap()
    bvg_d = nc.dram_tensor("bvg2", [P, CO], F32, kind="ExternalInput").ap()
    y_d = nc.dram_tensor("y", [C, L], F32, kind="ExternalOutput").ap()

    x3 = x_d.rearrange("(co ci) l -> ci co l", ci=P)      # [128, 2, 4096]
    y3 = y_d.rearrange("(co ci) l -> ci co l", ci=P)

    with tile.TileContext(nc) as tc:
        with tc.tile_pool(name="consts", bufs=1) as consts, \
             tc.tile_pool(name="big", bufs=1) as big, \
             tc.tile_pool(name="e", bufs=10) as e_pool, \
             tc.tile_pool(name="tmp", bufs=6) as tmp_pool, \
             tc.tile_pool(name="outp", bufs=4) as out_pool, \
             tc.tile_pool(name="pssp", bufs=2, space="PSUM") as ps_sp, \
             tc.tile_pool(name="psyh", bufs=3, space="PSUM") as ps_yh, \
             tc.tile_pool(name="psz", bufs=1, space="PSUM") as ps_z:

            # ---- warmup consts on gpsimd (earliest-starting engine) ----
            warm_w = consts.tile([P, P], BF16)     # full-array warmup lhsT
            nc.gpsimd.memset(warm_w, 1.0)
            warm_sb = consts.tile([P, LB], BF16)
            nc.gpsimd.memset(warm_sb, 1.0)

            # ---- input DMAs: early-critical fp8 x8/xh8 banks on the
            # hardware-DGE sync queue, then the fp32 x (residual only);
            # weights on the gpsimd queue.
            x_sb = big.tile([P, CO, L], F32)
            x8 = big.tile([P, CO, L], F8)
            xh8 = big.tile([P, CO, M], F8)
            g2_f = consts.tile([P, CO, C], F32)
            wv2_f = consts.tile([P, CO, C], F32)
            wbar_sb = consts.tile([P, CO], F32)
            bvg_sb = consts.tile([P, CO], F32)
            nc.sync.dma_start(out=g2_f, in_=g2_d)
            nc.sync.dma_start(out=wv2_f, in_=wv2_d)
            MB = M // NB                           # xh8 piece per bank
            for j in range(NB):
                sl = slice(j * LB, (j + 1) * LB)
                msl = slice(j * MB, (j + 1) * MB)
                if j == 0:
                    nc.sync.dma_start(out=x8[:, :, sl], in_=x8_d[:, :, sl])
                    nc.sync.dma_start(out=xh8[:, :, msl], in_=xh8_d[:, :, msl])
                else:
                    nc.sync.dma_start(out=xh8[:, :, msl], in_=xh8_d[:, :, msl])
                    nc.sync.dma_start(out=x8[:, :, sl], in_=x8_d[:, :, sl])
            for j in range(NB):
                sl = slice(j * LB, (j + 1) * LB)
                nc.sync.dma_start(out=x_sb[:, :, sl], in_=x3[:, :, sl])
            nc.gpsimd.dma_start(out=wbar_sb, in_=wbar_d)
            nc.gpsimd.dma_start(out=bvg_sb, in_=bvg_d)

            # full-array nonzero warmups: flip the PE HAM clock gate to 8/8
            # before the first real matmul (runs while the DMAs stream).
            for w in range(10):
                wp = ps_yh.tile([P, LB], F32, tag="yh", name=f"warm{w}")
                nc.tensor.matmul(wp, warm_w, warm_sb, start=True, stop=True)

            # ---- constants ----
            g2 = consts.tile([P, CO, C], F8)
            wv2 = consts.tile([P, CO, C], F8)
            nc.vector.tensor_copy(g2, g2_f)
            nc.vector.tensor_copy(wv2, wv2_f)
            ones2 = consts.tile([P, CO, 16], F8)   # DR lhsT for Z rows
            nc.vector.memset(ones2, 1.0)
            nshift = consts.tile([P, 1], F32)      # exp bias (-SHIFT)
            nc.vector.memset(nshift, -SHIFT)
            # tiny dummy exp: forces the ACT table load off the critical path
            dummy = consts.tile([1, 16], F32)
            nc.scalar.activation(dummy, warm_w[0:1, 0:16], AF.Exp)
            ones_row = consts.tile([1, P], BF16)   # 16x: recip yields 1/(16Z)
            nc.vector.memset(ones_row, 16.0)

            # ---- big persistent tensors ----
            t8 = big.tile([P, CO, L], F8)          # t'[c, l] = 64*(Gx+wbar)
            vt8 = big.tile([P, MA, 2, C], F8)      # v'[m, o] pair-interleaved

            def scores_pair(lt, a, pend):
                sl = slice(lt * LB, (lt + 1) * LB)
                sp2 = ps_sp.tile([P, CO, LB], F32, tag="sp",
                                 name=f"sp{lt}_{a}")
                for i in range(2):
                    mj = 2 * a + i
                    nc.tensor.matmul(
                        sp2[:, i, :], xh8[:, :, mj * P:(mj + 1) * P],
                        t8[:, :, sl], start=True, stop=True, perf_mode=DR)
                e2 = e_pool.tile([P, 2, LB], F8, tag="e", name=f"e{lt}_{a}")
                nc.scalar.activation(e2, sp2, AF.Exp,
                                     bias=nshift, scale=1.0 / SSCALE)
                pend[(lt, a)] = e2

            def consume_yh(a, yhp, e2):
                for oc in range(CO):
                    nc.tensor.matmul(
                        yhp[oc], vt8[:, a, :, oc * P:(oc + 1) * P], e2,
                        start=(a == 0), stop=(a == MA - 1), perf_mode=DR)

            def consume_z(a, zp, e2):
                nc.tensor.matmul(zp, ones2[:, :, :1], e2,
                                 start=(a == 0), stop=(a == MA - 1),
                                 perf_mode=DR)

            def consume_pair(a, zp, yhp, pend, lt=0):
                e2 = pend.pop((lt, a))
                consume_yh(a, yhp, e2)
                consume_z(a, zp, e2)

            def tile_tail(lt, zp, yhp):
                # b = 1/(16 Z) broadcast across partitions with a K=1 matmul
                # (the 1/16 in ones_row compensates the vt8 scale).
                sl = slice(lt * LB, (lt + 1) * LB)
                # Z -> SBUF (bf16), broadcast 16*Z across partitions with a
                # K=1 matmul, then one 128-lane reciprocal gives 1/(16 Z).
                zq = tmp_pool.tile([1, LB], BF16, tag="zq", name=f"zq{lt}")
                if lt == NB - 1:
                    nc.scalar.copy(zq, zp)     # scalar is idle at the end
                else:
                    nc.vector.tensor_copy(zq, zp)
                bp = ps_z.tile([P, LB], F32, tag="z", name=f"bp{lt}")
                nc.tensor.matmul(bp, ones_row, zq, start=True, stop=True)
                b_sb = tmp_pool.tile([P, LB], F32, tag="bsb")
                nc.vector.reciprocal_approx_fast(out=b_sb, in_=bp)
                o_sb = out_pool.tile([P, CO, LB], F32, tag="o")
                HB = LB // 2
                parts = [(0, 0), (1, 0), (0, 1), (1, 1)] \
                    if lt == NB - 1 else [(oc, None) for oc in range(CO)]
                for oc, lh in parts:
                    ls = slice(0, LB) if lh is None else \
                        slice(lh * HB, (lh + 1) * HB)
                    gsl = slice(lt * LB + ls.start, lt * LB + ls.stop)
                    u_sb = tmp_pool.tile([P, LB], F32, tag="t")
                    nc.vector.tensor_mul(u_sb[:, ls], yhp[oc][:, ls],
                                         b_sb[:, ls])
                    nc.vector.scalar_tensor_tensor(
                        out=o_sb[:, oc, ls], in0=u_sb[:, ls],
                        scalar=bvg_sb[:, oc:oc + 1],
                        in1=x_sb[:, oc, gsl], op0=ADD, op1=ADD)
                    eng = nc.sync if (lt == NB - 1 or (lt + oc) % 2) \
                        else nc.gpsimd
                    eng.dma_start(out=y3[:, oc, gsl], in_=o_sb[:, oc, ls])

            # ---- bank loop: projections + l-tile 0 attention ride the load
            def t_proj(j):
                # t projection for bank j (DR, K=256); drains split across
                # DVE and the scalar engine (Identity+bias), both fp8 out.
                sl = slice(j * LB, (j + 1) * LB)
                tp2 = ps_sp.tile([P, CO, LB], F32, tag="sp", name=f"tp{j}")
                for oc in range(CO):
                    nc.tensor.matmul(tp2[:, oc, :],
                                     g2[:, :, oc * P:(oc + 1) * P],
                                     x8[:, :, sl], start=True, stop=True,
                                     perf_mode=DR)
                if j == 0:
                    nc.scalar.add(t8[:, 0, sl], tp2[:, 0, :],
                                  wbar_sb[:, 0:1])
                    nc.scalar.add(t8[:, 1, sl], tp2[:, 1, :],
                                  wbar_sb[:, 1:2])
                else:
                    nc.vector.tensor_scalar_add(t8[:, 0, sl], tp2[:, 0, :],
                                                wbar_sb[:, 0:1])
                    nc.vector.tensor_scalar_add(t8[:, 1, sl], tp2[:, 1, :],
                                                wbar_sb[:, 1:2])

            def v_proj4(q):
                # v projection, four key chunks 4q..4q+3 -> vt8 pairs 2q,2q+1
                # (one 2-bank PSUM tile, one 1024-wide drain)
                vp2 = ps_sp.tile([P, CO, LB], F32, tag="sp", name=f"vp{q}")
                vflat = vp2.rearrange("p c l -> p (c l)")
                for k in range(4):
                    mj = 4 * q + k
                    nc.tensor.matmul(vflat[:, k * C:(k + 1) * C],
                                     xh8[:, :, mj * P:(mj + 1) * P],
                                     wv2, start=True, stop=True, perf_mode=DR)
                vdst = vt8[:, 2 * q:2 * q + 2, :, :]
                nc.vector.tensor_copy(
                    vdst.rearrange("p a i c -> p (a i c)"), vflat)

            # ---- global pair-stream: 64 score/exp pairs, consumers LAGP
            # behind, tile tails fired inline; bank-0 work and the t/v
            # projections ride the stream where each engine has slack.
            NP = NB * MA
            pend, zps, yhps = {}, {}, {}
            for g in range(NP + LAGP):
                if g < NP:
                    lt, a = divmod(g, MA)
                    if lt == 0 and a == 0:
                        t_proj(0)
                        v_proj4(0)          # vt8 pairs 0-1
                        v_proj4(1)          # vt8 pairs 2-3
                    scores_pair(lt, a, pend)
                    if lt == 0 and a == 1:
                        v_proj4(2)          # vt8 pairs 4-5 (slack slot)
                    if lt == 0 and a == 2:
                        v_proj4(3)          # vt8 pairs 6-7
                    if a == 1 and lt + 1 < NB:
                        t_proj(lt + 1)      # t8 for the NEXT tile
                c = g - LAGP
                if c >= 0:
                    lt, a = divmod(c, MA)
                    if a == 0:
                        zps[lt] = ps_z.tile([P, LB], F32, tag="z",
                                            name=f"zp{lt}")[0:1, :]
                        yhps[lt] = [ps_yh.tile([P, LB], F32, tag="yh",
                                               name=f"yh{lt}_{i}")
                                    for i in range(CO)]
                    if lt < NB - 1:
                        consume_pair(a, zps[lt], yhps[lt], pend, lt)
                        if a == MA - 1:
                            tile_tail(lt, zps.pop(lt), yhps.pop(lt))
                    else:
                        # final tile: Z leads yh by one pair so the 1/Z
                        # chain overlaps the last yh matmuls
                        consume_z(a, zps[lt], pend[(lt, a)])
                        if a > 0:
                            consume_yh(a - 1, yhps[lt], pend.pop((lt, a - 1)))
            lt = NB - 1
            consume_yh(MA - 1, yhps[lt], pend.pop((lt, MA - 1)))
            tile_tail(lt, zps.pop(lt), yhps.pop(lt))

    nc.compile()
    return nc


def _get_nc():
    if "nc" not in _CACHE:
        _CACHE["nc"] = _build()
    return _CACHE["nc"]


def _chunk_pairs(a):   # [d, c] -> [di, dj, c] with d = dj*128 + di
    return np.ascontiguousarray(
        a.reshape(CO, P, -1).transpose(1, 0, 2))


def kernel(x, Wq, bq, Wk, bk, Wv, bv, attn_gate, _run_kwargs=None):
    x = np.asarray(x, dtype=np.float32)
    Wq = np.asarray(Wq, dtype=np.float32)
    Wk = np.asarray(Wk, dtype=np.float32)
    Wv = np.asarray(Wv, dtype=np.float32)
    bq = np.asarray(bq, dtype=np.float32)
    bv = np.asarray(bv, dtype=np.float32)
    gate = float(np.tanh(np.asarray(attn_gate, dtype=np.float64))[0])

    s = 1.0 / np.sqrt(np.float64(C))
    sc_s = np.float64(SSCALE) * INV_SQRT2 * s          # scores-path scale
    # G-fusion: t = (Wk^T Wq) x + Wk^T bq, scaled by 64/sqrt(2C); bk cancels.
    Gp = (Wk.astype(np.float64).T @ Wq.astype(np.float64)) * sc_s
    wbar = (Wk.astype(np.float64).T @ bq.astype(np.float64)) * sc_s
    wvp = Wv.astype(np.float64).T * (INV_SQRT2 * gate * 16.0)

    g2 = _chunk_pairs(Gp.T).astype(np.float32)         # lhsT[d, c] = Gp[c, d]
    wv2 = _chunk_pairs(wvp).astype(np.float32)         # rhs[c, o]
    wbar2 = np.ascontiguousarray(wbar.reshape(CO, P).T).astype(np.float32)
    bvg2 = np.ascontiguousarray(
        (bv.astype(np.float64) * gate).reshape(CO, P).T).astype(np.float32)

    def q8(a):   # TRN fp8e4 (ml_dtypes e4m3, max +-240)
        return np.clip(a, -240.0, 240.0).astype(E4)

    nc = _get_nc()
    in_maps = []
    for b in range(B):
        xb = x[b]
        x8 = np.ascontiguousarray(_chunk_pairs(q8(xb)))
        xh8 = np.ascontiguousarray(_chunk_pairs(q8(xb[:, 0::2] - xb[:, 1::2])))
        in_maps.append({
            "x": np.ascontiguousarray(xb), "x8": x8, "xh8": xh8,
            "g2": g2, "wv2": wv2, "wbar2": wbar2, "bvg2": bvg2,
        })
    res = bass_utils.run_bass_kernel_spmd(
        nc, in_maps, core_ids=list(range(B)), **(_run_kwargs or {}))
    out = np.stack([res.results[b]["y"] for b in range(B)]).astype(np.float32)
    if _run_kwargs:
        kernel.last_results = res
    return out
